# revision 15
# baseline (speedup 1.0000x reference)
"""Trainium2 Bass kernel for the SNN Leaky-Integrate-Fire problem.

Pipeline (per core, pure data-parallel over batch, everything on the DVE):
  cn   = -(x @ W1.T + b1)                     [128, 2048]  (6 tensor_tensor ops)
  scan: 100 LIF steps on negated state n = -mem,
        n' = beta*n + cn + (n < -1),
        fused 2 steps per custom-DVE instruction (50 instructions total):
          1x LIF2_B0  (steps 2,3; n1 = cn, single stream)
          48x LIF2    (steps 4..99)
          1x LIF1_SPK (step 100 + spike emit)
  out  = spk @ W2.T + b2                      [128, 192]   (3x TT+reduce)

The custom DVE ops are registered at import time (runtime-patch of
concourse.dve_ops.OPS); each op's per-stage fp32 rounding matches the
reference's fl(fl(fl(beta*m)+cur1)-h) sequence bit-for-bit.

Layout per core: 8192 rows; logical row r = ch*128 + p lives at
partition p, free block ch; scan free index = ch*32 + h. Host feeds
xr [128, 192] with xr[p, i*64+ch] = x[ch*128+p, i] and inverse-permutes
the output rows.
"""
import sys

sys.path.insert(0, "/opt/trn_rl_repo")

import numpy as np

import concourse.bacc as bacc
import concourse.tile as tile
from concourse import mybir
from concourse import dve_ops as dvo
from concourse.dve_spec import Spec, Src0, Src1, C0, C1, lower, _has_src1
from concourse.dve_uop import DveOpSpec
from concourse.bass_utils import run_bass_kernel_spmd

F32 = mybir.dt.float32
BF16 = mybir.dt.bfloat16
ALU = mybir.AluOpType

# problem constants (hardcoded per contract)
B, N_IN, N_HID, N_OUT = 65536, 3, 32, 3
NUM_STEPS, BETA, THR = 100, 0.9, 1.0
N_CORES = 8
BC = B // N_CORES          # rows per core = 8192
P = 128                    # partitions
NCH = BC // P              # 128-row chunks per core = 64
FREE = NCH * N_HID         # scan free size = 2048

# const block layout (replicated across partitions):
# [negW1 f0|f1|f2 (3*32)  negb1(32)  w2 o0|o1|o2 (3*32)  b2(3)]
NW1_OFF, NB1_OFF, W2_OFF, B2_OFF = 0, 96, 128, 224
WB_COLS = 256


# --- custom DVE op registration (runtime-patch of dve_ops.OPS) -------------


def _nstep(n, cn, b, th):
    return ((n * np.float32(b) + cn).astype(np.float32) + (n < th)).astype(
        np.float32
    )


def _ref_lif2_b0(in0, in1, s0, s1, imm2):
    n = in0.astype(np.float32)
    return _nstep(_nstep(n, n, s0, s1), n, s0, s1)


def _ref_lif2(in0, in1, s0, s1, imm2):
    cn = in1.astype(np.float32)
    return _nstep(_nstep(in0.astype(np.float32), cn, s0, s1), cn, s0, s1)


def _ref_lif1_spk(in0, in1, s0, s1, imm2):
    n = _nstep(in0.astype(np.float32), in1.astype(np.float32), s0, s1)
    return (n < np.float32(s1)).astype(np.float32)


def _register_op(name, spec):
    for o in dvo.OPS:
        if o.name == name:
            return o
    row = dvo._CUSTOM_DVE_ROW_BASE + len(dvo.OPS)
    dvo._SUB_OPCODE_FOR_NAME[name] = row
    uops = lower(spec, ver="v3")
    sha = DveOpSpec(name=name, opcode=row, uops=uops, rd1_en=_has_src1(spec)).sha(
        "v3"
    )
    op = dvo.DveOp(name, spec, subdim=False, uops_sha={"v3": sha})
    dvo.OPS.append(op)
    dvo.CUSTOM_DVE_SPECS[name] = spec
    return op


def _make_ops():
    n2_ = (Src0 * C0 + Src0) + (Src0 < C1)
    n3_ = (n2_ * C0 + Src0) + (n2_ < C1)
    b0 = _register_op("LIF2_B0_ANT", Spec(body=n3_, reference=_ref_lif2_b0))
    s1_ = (Src0 * C0 + Src1) + (Src0 < C1)
    s2_ = (s1_ * C0 + Src1) + (s1_ < C1)
    l2 = _register_op("LIF2_ANT", Spec(body=s2_, reference=_ref_lif2))
    spk = _register_op(
        "LIF1_SPK_ANT", Spec(body=(s1_ < C1), reference=_ref_lif1_spk)
    )
    return b0, l2, spk


LIF2_B0, LIF2, LIF1_SPK = _make_ops()


def build(nc, num_steps=NUM_STEPS):
    xr_d = nc.dram_tensor("xr", [P, N_IN * NCH], F32, kind="ExternalInput")
    wb_d = nc.dram_tensor("wb", [P, WB_COLS], F32, kind="ExternalInput")
    w2b_d = nc.dram_tensor("w2b", [P, 96], BF16, kind="ExternalInput")
    y_d = nc.dram_tensor("y", [P, NCH * N_OUT], BF16, kind="ExternalOutput")

    dve = nc.vector

    def h3(ap):
        return ap.rearrange("p (i h) -> p i h", h=N_HID)

    def cbc(wt, off):
        # [P, 32] const slice -> [P, NCH, 32] broadcast view (middle stride-0)
        return wt[:, off : off + 32].unsqueeze(1).broadcast_to([P, NCH, N_HID])

    with tile.TileContext(nc) as tc:
        with tc.tile_pool(name="pool", bufs=1) as pool:
            xt = pool.tile([P, N_IN * NCH], F32, tag="xt")
            nc.sync.dma_start(xt[:], xr_d[:])
            wt = pool.tile([P, WB_COLS], F32, tag="wt")
            nc.sync.dma_start(wt[:], wb_d[:])
            w2t = pool.tile([P, 96], BF16, tag="w2t")
            nc.sync.dma_start(w2t[:], w2b_d[:])

            cn = pool.tile([P, FREE], F32, tag="cn")
            na = pool.tile([P, FREE], F32, tag="na")
            nb = pool.tile([P, FREE], F32, tag="nb")
            spkb = pool.tile([P, FREE], BF16, tag="spkb")
            scrb = pool.tile([P, FREE], BF16, tag="scrb")
            ot = pool.tile([P, NCH * N_OUT], BF16, tag="ot")

            def xbc(i):
                # x feature i: [P, NCH] -> [P, NCH, 32] (inner stride-0)
                return (
                    xt[:, i * NCH : (i + 1) * NCH]
                    .unsqueeze(2)
                    .broadcast_to([P, NCH, N_HID])
                )

            # --- cn = -(x @ W1.T + b1): 6 TT ops on DVE ---
            dve.tensor_tensor(h3(na[:]), xbc(0), cbc(wt, NW1_OFF), ALU.mult)
            dve.tensor_tensor(h3(nb[:]), xbc(1), cbc(wt, NW1_OFF + 32), ALU.mult)
            dve.tensor_tensor(na[:], na[:], nb[:], ALU.add)
            dve.tensor_tensor(h3(nb[:]), xbc(2), cbc(wt, NW1_OFF + 64), ALU.mult)
            dve.tensor_tensor(na[:], na[:], nb[:], ALU.add)
            dve.tensor_tensor(h3(cn[:]), h3(na[:]), cbc(wt, NB1_OFF), ALU.add)

            # --- scan: steps 2..3 seeded from n1 = cn, then 2 steps/instr.
            # Two independent half-column chains interleaved so each
            # instruction's input is 2 instructions old (hides the
            # dependent-write ack latency).
            H = FREE // 2
            halves = [(cn[:, :H], na[:, :H], nb[:, :H]),
                      (cn[:, H:], na[:, H:], nb[:, H:])]
            for c_, a_, b_ in halves:
                dve._custom_dve(LIF2_B0, out=a_, in0=c_, s0=BETA, s1=-THR)
            states = [[a_, b_] for c_, a_, b_ in halves]
            n_lif2 = (num_steps - 4) // 2  # steps 4..99 -> 48 instructions
            for _ in range(n_lif2):
                for hi, (c_, a_, b_) in enumerate(halves):
                    cur_, nxt_ = states[hi]
                    dve._custom_dve(
                        LIF2, out=nxt_, in0=cur_, in1=c_, s0=BETA, s1=-THR
                    )
                    states[hi] = [nxt_, cur_]
            # step 100 + spike (emitted as bf16: spikes are 0/1, exact)
            for hi, (c_, a_, b_) in enumerate(halves):
                cur_, nxt_ = states[hi]
                dve._custom_dve(
                    LIF1_SPK, out=spkb[:, hi * H : (hi + 1) * H], in0=cur_,
                    in1=c_, s0=BETA, s1=-THR,
                )
            assert n_lif2 % 2 == 0
            spk = spkb

            # --- fc2: out[:, (i,o)] = sum_h spk * W2[o] + b2[o] ---
            # o-major layout: ot[p, o*NCH + i] so each reduce output slice is
            # contiguous (keeps the 2-byte packed requirement for DVE 2x mode)
            ov = ot[:].rearrange("p (o i) -> p o i", o=N_OUT)

            def w2bc(o):
                return (
                    w2t[:, 32 * o : 32 * (o + 1)]
                    .unsqueeze(1)
                    .broadcast_to([P, NCH, N_HID])
                )

            with nc.allow_low_precision(reason="fc2 in bf16: |err| ~4e-3 rel, gate 2e-2"):
                for o in range(N_OUT):
                    dve.tensor_tensor(h3(scrb[:]), h3(spk[:]), w2bc(o), ALU.mult)
                    dve.tensor_reduce(
                        ov[:, o : o + 1, :], h3(scrb[:]), mybir.AxisListType.X,
                        ALU.add,
                    )
                    dve.tensor_scalar(
                        ov[:, o : o + 1, :], ov[:, o : o + 1, :],
                        wt[:, B2_OFF + o : B2_OFF + o + 1], None, ALU.add,
                    )

            nc.sync.dma_start(y_d[:], ot[:])
    return nc


_CACHE = {}


def _get_program():
    if "nc" not in _CACHE:
        nc = bacc.Bacc("TRN2", target_bir_lowering=False, debug=False,
                       num_devices=N_CORES)
        build(nc)
        nc.compile()
        _CACHE["nc"] = nc
    return _CACHE["nc"]


def make_wb(W1, b1, W2, b2):
    wb = np.zeros((P, WB_COLS), dtype=np.float32)
    for i in range(N_IN):
        wb[:, NW1_OFF + 32 * i : NW1_OFF + 32 * (i + 1)] = -W1[:, i]
    wb[:, NB1_OFF : NB1_OFF + 32] = -b1
    wb[:, W2_OFF : W2_OFF + 96] = np.ascontiguousarray(W2).reshape(-1)
    wb[:, B2_OFF : B2_OFF + 3] = b2
    return wb


def kernel(x, W1, b1, W2, b2):
    import ml_dtypes

    x = np.asarray(x, dtype=np.float32)
    W1, b1, W2, b2 = (np.asarray(a, dtype=np.float32) for a in (W1, b1, W2, b2))
    wb = make_wb(W1, b1, W2, b2)
    w2b = np.broadcast_to(
        np.ascontiguousarray(W2).reshape(-1).astype(ml_dtypes.bfloat16), (P, 96)
    )
    w2b = np.ascontiguousarray(w2b)
    nc = _get_program()
    in_maps = []
    for i in range(N_CORES):
        xs = x[i * BC : (i + 1) * BC].reshape(NCH, P, N_IN)
        xr = np.ascontiguousarray(xs.transpose(1, 2, 0).reshape(P, N_IN * NCH))
        in_maps.append({"xr": xr, "wb": wb, "w2b": w2b})
    kwargs = dict(_CACHE.get("run_kwargs") or {})
    res = run_bass_kernel_spmd(nc, in_maps, core_ids=list(range(N_CORES)), **kwargs)
    _CACHE["last_results"] = res
    # y rows are stored permuted: col ch*3+o of partition p <-> logical row ch*128+p
    out = np.empty((B, N_OUT), dtype=np.float32)
    for i in range(N_CORES):
        yc = res.results[i]["y"].astype(np.float32).reshape(P, N_OUT, NCH)
        out[i * BC : (i + 1) * BC] = yc.transpose(2, 0, 1).reshape(BC, N_OUT)
    return out


# revision 19
# speedup vs baseline: 1.0265x; 1.0265x over previous
"""Trainium2 Bass kernel for the SNN Leaky-Integrate-Fire problem.

Pipeline (per core, pure data-parallel over batch, everything on the DVE):
  cn   = -(x @ W1.T + b1)                     [128, 2048]  (6 tensor_tensor ops)
  scan: 100 LIF steps on negated state n = -mem,
        n' = beta*n + cn + (n < -1),
        fused 2 steps per custom-DVE instruction (50 instructions total):
          1x LIF2_B0  (steps 2,3; n1 = cn, single stream)
          48x LIF2    (steps 4..99)
          1x LIF1_SPK (step 100 + spike emit)
  out  = spk @ W2.T + b2                      [128, 192]   (3x TT+reduce)

The custom DVE ops are registered at import time (runtime-patch of
concourse.dve_ops.OPS); each op's per-stage fp32 rounding matches the
reference's fl(fl(fl(beta*m)+cur1)-h) sequence bit-for-bit.

Layout per core: 8192 rows; logical row r = ch*128 + p lives at
partition p, free block ch; scan free index = ch*32 + h. Host feeds
xr [128, 192] with xr[p, i*64+ch] = x[ch*128+p, i] and inverse-permutes
the output rows.
"""
import sys

sys.path.insert(0, "/opt/trn_rl_repo")

import numpy as np

import concourse.bacc as bacc
import concourse.tile as tile
from concourse import mybir
from concourse import dve_ops as dvo
from concourse.dve_spec import Spec, Src0, Src1, C0, C1, lower, _has_src1
from concourse.dve_uop import DveOpSpec
from concourse.bass_utils import run_bass_kernel_spmd

F32 = mybir.dt.float32
F16 = mybir.dt.float16
ALU = mybir.AluOpType

# problem constants (hardcoded per contract)
B, N_IN, N_HID, N_OUT = 65536, 3, 32, 3
NUM_STEPS, BETA, THR = 100, 0.9, 1.0
N_CORES = 8
BC = B // N_CORES          # rows per core = 8192
P = 128                    # partitions
NCH = BC // P              # 128-row chunks per core = 64
FREE = NCH * N_HID         # scan free size = 2048

# const block layout (replicated across partitions):
# [negW1 f0|f1|f2 (3*32)  negb1(32)  w2 o0|o1|o2 (3*32)  b2(3)]
NW1_OFF, NB1_OFF, W2_OFF, B2_OFF = 0, 96, 128, 224
WB_COLS = 256


# --- custom DVE op registration (runtime-patch of dve_ops.OPS) -------------


def _nstep(n, cn, b, th):
    return ((n * np.float32(b) + cn).astype(np.float32) + (n < th)).astype(
        np.float32
    )


def _ref_lif2_b0(in0, in1, s0, s1, imm2):
    n = in0.astype(np.float32)
    return _nstep(_nstep(n, n, s0, s1), n, s0, s1)


def _ref_lif2(in0, in1, s0, s1, imm2):
    cn = in1.astype(np.float32)
    return _nstep(_nstep(in0.astype(np.float32), cn, s0, s1), cn, s0, s1)


def _ref_lif1_spk(in0, in1, s0, s1, imm2):
    n = _nstep(in0.astype(np.float32), in1.astype(np.float32), s0, s1)
    return (n < np.float32(s1)).astype(np.float32)


def _register_op(name, spec):
    for o in dvo.OPS:
        if o.name == name:
            return o
    row = dvo._CUSTOM_DVE_ROW_BASE + len(dvo.OPS)
    dvo._SUB_OPCODE_FOR_NAME[name] = row
    uops = lower(spec, ver="v3")
    sha = DveOpSpec(name=name, opcode=row, uops=uops, rd1_en=_has_src1(spec)).sha(
        "v3"
    )
    op = dvo.DveOp(name, spec, subdim=False, uops_sha={"v3": sha})
    dvo.OPS.append(op)
    dvo.CUSTOM_DVE_SPECS[name] = spec
    return op


def _make_ops():
    n2_ = (Src0 * C0 + Src0) + (Src0 < C1)
    n3_ = (n2_ * C0 + Src0) + (n2_ < C1)
    b0 = _register_op("LIF2_B0_ANT", Spec(body=n3_, reference=_ref_lif2_b0))
    s1_ = (Src0 * C0 + Src1) + (Src0 < C1)
    s2_ = (s1_ * C0 + Src1) + (s1_ < C1)
    l2 = _register_op("LIF2_ANT", Spec(body=s2_, reference=_ref_lif2))
    spk = _register_op(
        "LIF1_SPK_ANT", Spec(body=(s1_ < C1), reference=_ref_lif1_spk)
    )
    return b0, l2, spk


LIF2_B0, LIF2, LIF1_SPK = _make_ops()


def build(nc, num_steps=NUM_STEPS):
    xr_d = nc.dram_tensor("xr", [P, N_IN * NCH], F32, kind="ExternalInput")
    wb_d = nc.dram_tensor("wb", [P, WB_COLS], F32, kind="ExternalInput")
    w2b_d = nc.dram_tensor("w2b", [P, 96], F16, kind="ExternalInput")
    y_d = nc.dram_tensor("y", [P, NCH * N_OUT], F16, kind="ExternalOutput")

    dve = nc.vector

    def h3(ap):
        return ap.rearrange("p (i h) -> p i h", h=N_HID)

    def cbc(wt, off):
        # [P, 32] const slice -> [P, NCH, 32] broadcast view (middle stride-0)
        return wt[:, off : off + 32].unsqueeze(1).broadcast_to([P, NCH, N_HID])

    with tile.TileContext(nc) as tc:
        with tc.tile_pool(name="pool", bufs=1) as pool:
            xt = pool.tile([P, N_IN * NCH], F32, tag="xt")
            nc.sync.dma_start(xt[:], xr_d[:])
            wt = pool.tile([P, WB_COLS], F32, tag="wt")
            nc.sync.dma_start(wt[:], wb_d[:])
            w2t = pool.tile([P, 96], F16, tag="w2t")
            nc.sync.dma_start(w2t[:], w2b_d[:])

            cn = pool.tile([P, FREE], F32, tag="cn")
            na = pool.tile([P, FREE], F32, tag="na")
            nb = pool.tile([P, FREE], F32, tag="nb")
            spkb = pool.tile([P, FREE], F16, tag="spkb")
            prod = pool.tile([P, N_OUT * FREE], F16, tag="prod")
            scrc = pool.tile([P, N_OUT * FREE // 2], F16, tag="scrc")
            ot = pool.tile([P, NCH * N_OUT], F16, tag="ot")

            def xbc(i):
                # x feature i: [P, NCH] -> [P, NCH, 32] (inner stride-0)
                return (
                    xt[:, i * NCH : (i + 1) * NCH]
                    .unsqueeze(2)
                    .broadcast_to([P, NCH, N_HID])
                )

            # --- cn = -(x @ W1.T + b1): 6 TT ops on DVE ---
            dve.tensor_tensor(h3(na[:]), xbc(0), cbc(wt, NW1_OFF), ALU.mult)
            dve.tensor_tensor(h3(nb[:]), xbc(1), cbc(wt, NW1_OFF + 32), ALU.mult)
            dve.tensor_tensor(na[:], na[:], nb[:], ALU.add)
            dve.tensor_tensor(h3(nb[:]), xbc(2), cbc(wt, NW1_OFF + 64), ALU.mult)
            dve.tensor_tensor(na[:], na[:], nb[:], ALU.add)
            dve.tensor_tensor(h3(cn[:]), h3(na[:]), cbc(wt, NB1_OFF), ALU.add)

            # --- scan: steps 2..3 seeded from n1 = cn, then 2 steps/instr.
            # Two independent half-column chains interleaved so each
            # instruction's input is 2 instructions old (hides the
            # dependent-write ack latency).
            H = FREE // 2
            halves = [(cn[:, :H], na[:, :H], nb[:, :H]),
                      (cn[:, H:], na[:, H:], nb[:, H:])]
            for c_, a_, b_ in halves:
                dve._custom_dve(LIF2_B0, out=a_, in0=c_, s0=BETA, s1=-THR)
            states = [[a_, b_] for c_, a_, b_ in halves]
            n_lif2 = (num_steps - 4) // 2  # steps 4..99 -> 48 instructions
            for _ in range(n_lif2):
                for hi, (c_, a_, b_) in enumerate(halves):
                    cur_, nxt_ = states[hi]
                    dve._custom_dve(
                        LIF2, out=nxt_, in0=cur_, in1=c_, s0=BETA, s1=-THR
                    )
                    states[hi] = [nxt_, cur_]
            # step 100 + spike (emitted as bf16: spikes are 0/1, exact)
            for hi, (c_, a_, b_) in enumerate(halves):
                cur_, nxt_ = states[hi]
                dve._custom_dve(
                    LIF1_SPK, out=spkb[:, hi * H : (hi + 1) * H], in0=cur_,
                    in1=c_, s0=BETA, s1=-THR,
                )
            assert n_lif2 % 2 == 0
            spk = spkb

            # --- fc2: out[p, o*NCH+i] = sum_h spk * W2[o,h] + b2[o] ---
            # fp16 products (2x DVE mode), then one shared pairwise add-tree
            # over all 3 output channels (reduce has no 2x mode; TT does).
            def w2bc(o):
                return (
                    w2t[:, 32 * o : 32 * (o + 1)]
                    .unsqueeze(1)
                    .broadcast_to([P, NCH, N_HID])
                )

            G = N_OUT * NCH  # 192 groups of 32
            with nc.allow_low_precision(
                reason="fc2 in fp16: |err| ~1e-3 rel, gate 2e-2"
            ):
                for o in range(N_OUT):
                    dve.tensor_tensor(
                        h3(prod[:, o * FREE : (o + 1) * FREE]), h3(spk[:]),
                        w2bc(o), ALU.mult,
                    )
                src, dst = prod, scrc
                w = N_HID
                while w > 1:
                    hw = w // 2
                    sv = src[:, : G * w].rearrange("p (g h) -> p g h", h=w)
                    if hw == 1:
                        dv = ot[:].unsqueeze(2)
                    else:
                        dv = dst[:, : G * hw].rearrange(
                            "p (g h) -> p g h", h=hw
                        )
                    dve.tensor_tensor(
                        dv, sv[:, :, :hw], sv[:, :, hw:], ALU.add
                    )
                    src, dst = dst, src
                    w = hw
                for o in range(N_OUT):
                    sl = ot[:, o * NCH : (o + 1) * NCH]
                    dve.tensor_scalar(
                        sl, sl, wt[:, B2_OFF + o : B2_OFF + o + 1], None,
                        ALU.add,
                    )

            nc.sync.dma_start(y_d[:], ot[:])
    return nc


_CACHE = {}


def _get_program():
    if "nc" not in _CACHE:
        nc = bacc.Bacc("TRN2", target_bir_lowering=False, debug=False,
                       num_devices=N_CORES)
        build(nc)
        nc.compile()
        _CACHE["nc"] = nc
    return _CACHE["nc"]


def make_wb(W1, b1, W2, b2):
    wb = np.zeros((P, WB_COLS), dtype=np.float32)
    for i in range(N_IN):
        wb[:, NW1_OFF + 32 * i : NW1_OFF + 32 * (i + 1)] = -W1[:, i]
    wb[:, NB1_OFF : NB1_OFF + 32] = -b1
    wb[:, W2_OFF : W2_OFF + 96] = np.ascontiguousarray(W2).reshape(-1)
    wb[:, B2_OFF : B2_OFF + 3] = b2
    return wb


def kernel(x, W1, b1, W2, b2):
    import ml_dtypes

    x = np.asarray(x, dtype=np.float32)
    W1, b1, W2, b2 = (np.asarray(a, dtype=np.float32) for a in (W1, b1, W2, b2))
    wb = make_wb(W1, b1, W2, b2)
    w2b = np.broadcast_to(
        np.ascontiguousarray(W2).reshape(-1).astype(np.float16), (P, 96)
    )
    w2b = np.ascontiguousarray(w2b)
    nc = _get_program()
    in_maps = []
    for i in range(N_CORES):
        xs = x[i * BC : (i + 1) * BC].reshape(NCH, P, N_IN)
        xr = np.ascontiguousarray(xs.transpose(1, 2, 0).reshape(P, N_IN * NCH))
        in_maps.append({"xr": xr, "wb": wb, "w2b": w2b})
    kwargs = dict(_CACHE.get("run_kwargs") or {})
    res = run_bass_kernel_spmd(nc, in_maps, core_ids=list(range(N_CORES)), **kwargs)
    _CACHE["last_results"] = res
    # y rows are stored permuted: col ch*3+o of partition p <-> logical row ch*128+p
    out = np.empty((B, N_OUT), dtype=np.float32)
    for i in range(N_CORES):
        yc = res.results[i]["y"].astype(np.float32).reshape(P, N_OUT, NCH)
        out[i * BC : (i + 1) * BC] = yc.transpose(2, 0, 1).reshape(BC, N_OUT)
    return out


# revision 21
# speedup vs baseline: 1.0853x; 1.0573x over previous
"""Trainium2 Bass kernel for the SNN Leaky-Integrate-Fire problem.

Layout per core (8192 rows): partition p = r4*32 + h (r4 = row-group 0..3,
h = hidden 0..31); free index j in [0, 2048); logical row r = r4*2048 + j.
This puts the hidden dim in the partition axis, so fc1/fc2 become PE
matmuls over the partition (k) dim and all h-dependent constants are
per-partition scalars:

  PE:  cn = -(x @ W1.T + b1)  via stationary S1[13,128] (-W1/-b1 baked in,
       ones row for the bias), moving xj[13,2048] -> PSUM [128,2048] fp32
  ACT: copy cn PSUM -> SBUF
  DVE: 100-step LIF scan on negated state n = -mem, n' = beta*n+cn+(n<-1),
       2 steps fused per custom-DVE instruction (50 instructions), spike
       emitted as fp16
  PE:  out = spk @ W2.T  via stationary S2[128,12] -> PSUM [12,2048]
  ACT: + b2 and evict to SBUF, DMA out

The custom DVE ops are registered at import time (runtime-patch of
concourse.dve_ops.OPS); their per-stage fp32 rounding matches the
reference's fl(fl(fl(beta*m)+cur1)-h) sequence bit-for-bit.
"""
import sys

sys.path.insert(0, "/opt/trn_rl_repo")

import numpy as np

import concourse.bacc as bacc
import concourse.tile as tile
from concourse import mybir
from concourse import dve_ops as dvo
from concourse.dve_spec import Spec, Src0, Src1, C0, C1, lower, _has_src1
from concourse.dve_uop import DveOpSpec
from concourse.bass_utils import run_bass_kernel_spmd

F32 = mybir.dt.float32
F16 = mybir.dt.float16
ALU = mybir.AluOpType
AF = mybir.ActivationFunctionType

# problem constants (hardcoded per contract)
B, N_IN, N_HID, N_OUT = 65536, 3, 32, 3
NUM_STEPS, BETA, THR = 100, 0.9, 1.0
N_CORES = 8
BC = B // N_CORES          # rows per core = 8192
P = 128                    # partitions
NG = 4                     # row groups per core
J = BC // NG               # rows per group = free size = 2048
KM = N_IN * NG + 1         # moving rows for fc1 = 13 (x features x groups + ones)
QO = N_OUT * NG            # fc2 output partitions = 12


# --- custom DVE op registration (runtime-patch of dve_ops.OPS) -------------


def _nstep(n, cn, b, th):
    return ((n * np.float32(b) + cn).astype(np.float32) + (n < th)).astype(
        np.float32
    )


def _ref_lif2_b0(in0, in1, s0, s1, imm2):
    n = in0.astype(np.float32)
    return _nstep(_nstep(n, n, s0, s1), n, s0, s1)


def _ref_lif2(in0, in1, s0, s1, imm2):
    cn = in1.astype(np.float32)
    return _nstep(_nstep(in0.astype(np.float32), cn, s0, s1), cn, s0, s1)


def _ref_lif1_spk(in0, in1, s0, s1, imm2):
    n = _nstep(in0.astype(np.float32), in1.astype(np.float32), s0, s1)
    return (n < np.float32(s1)).astype(np.float32)


def _register_op(name, spec):
    for o in dvo.OPS:
        if o.name == name:
            return o
    row = dvo._CUSTOM_DVE_ROW_BASE + len(dvo.OPS)
    dvo._SUB_OPCODE_FOR_NAME[name] = row
    uops = lower(spec, ver="v3")
    sha = DveOpSpec(name=name, opcode=row, uops=uops, rd1_en=_has_src1(spec)).sha(
        "v3"
    )
    op = dvo.DveOp(name, spec, subdim=False, uops_sha={"v3": sha})
    dvo.OPS.append(op)
    dvo.CUSTOM_DVE_SPECS[name] = spec
    return op


def _make_ops():
    n2_ = (Src0 * C0 + Src0) + (Src0 < C1)
    n3_ = (n2_ * C0 + Src0) + (n2_ < C1)
    b0 = _register_op("LIF2_B0_ANT", Spec(body=n3_, reference=_ref_lif2_b0))
    s1_ = (Src0 * C0 + Src1) + (Src0 < C1)
    s2_ = (s1_ * C0 + Src1) + (s1_ < C1)
    l2 = _register_op("LIF2_ANT", Spec(body=s2_, reference=_ref_lif2))
    spk = _register_op(
        "LIF1_SPK_ANT", Spec(body=(s1_ < C1), reference=_ref_lif1_spk)
    )
    return b0, l2, spk


LIF2_B0, LIF2, LIF1_SPK = _make_ops()


def build(nc, num_steps=NUM_STEPS):
    xj_d = nc.dram_tensor("xj", [KM, J], F32, kind="ExternalInput")
    s1_d = nc.dram_tensor("s1", [KM, P], F32, kind="ExternalInput")
    s2_d = nc.dram_tensor("s2", [P, QO], F16, kind="ExternalInput")
    b2_d = nc.dram_tensor("b2c", [QO, 1], F32, kind="ExternalInput")
    y_d = nc.dram_tensor("y", [QO, J], F32, kind="ExternalOutput")

    dve = nc.vector

    with tile.TileContext(nc) as tc:
        with tc.tile_pool(name="pool", bufs=1) as pool, \
             tc.tile_pool(name="ps", bufs=1, space="PSUM") as psp:
            xjt = pool.tile([KM, J], F32, tag="xjt")
            nc.sync.dma_start(xjt[:], xj_d[:])
            s1t = pool.tile([KM, P], F32, tag="s1t")
            nc.sync.dma_start(s1t[:], s1_d[:])
            s2t = pool.tile([P, QO], F16, tag="s2t")
            nc.sync.dma_start(s2t[:], s2_d[:])
            b2t = pool.tile([QO, 1], F32, tag="b2t")
            nc.sync.dma_start(b2t[:], b2_d[:])

            cnp = psp.tile([P, J], F32, tag="cnp")
            cn = pool.tile([P, J], F32, tag="cn")
            na = pool.tile([P, J], F32, tag="na")
            nb = pool.tile([P, J], F32, tag="nb")
            spkb = pool.tile([P, J], F16, tag="spkb")
            yp = psp.tile([QO, J], F32, tag="yp")
            yt = pool.tile([QO, J], F32, tag="yt")

            # --- fc1 on PE: cn = -(x @ W1.T + b1) into PSUM ---
            NB = 4  # 512-col PSUM banks
            for b in range(NB):
                sl = slice(b * (J // NB), (b + 1) * (J // NB))
                nc.tensor.matmul(
                    cnp[:, sl], s1t[:], xjt[:, sl], start=True, stop=True
                )
            # ACT copies cn PSUM -> SBUF while DVE's first scan op reads PSUM
            nc.scalar.copy(cn[:], cnp[:])

            # --- scan: steps 2..3 seeded from n1 = cn, then 2 steps/instr.
            # Two independent half-column chains interleaved to hide the
            # dependent-write ack latency. First op reads cn from PSUM so it
            # does not wait for the ACT copy.
            H = J // 2
            halves = [
                (cnp[:, :H], cn[:, :H], na[:, :H], nb[:, :H]),
                (cnp[:, H:], cn[:, H:], na[:, H:], nb[:, H:]),
            ]
            for cp_, c_, a_, b_ in halves:
                dve._custom_dve(LIF2_B0, out=a_, in0=cp_, s0=BETA, s1=-THR)
            states = [[a_, b_] for cp_, c_, a_, b_ in halves]
            n_lif2 = (num_steps - 4) // 2  # steps 4..99 -> 48 instructions
            for _ in range(n_lif2):
                for hi, (cp_, c_, a_, b_) in enumerate(halves):
                    cur_, nxt_ = states[hi]
                    dve._custom_dve(
                        LIF2, out=nxt_, in0=cur_, in1=c_, s0=BETA, s1=-THR
                    )
                    states[hi] = [nxt_, cur_]
            # step 100 + spike (fp16: spikes are 0/1, exact)
            for hi, (cp_, c_, a_, b_) in enumerate(halves):
                cur_, nxt_ = states[hi]
                dve._custom_dve(
                    LIF1_SPK, out=spkb[:, hi * H : (hi + 1) * H], in0=cur_,
                    in1=c_, s0=BETA, s1=-THR,
                )

            # --- fc2 on PE: yp[(o,r4), j] = sum_h W2[o,h] spk[(r4,h), j] ---
            with nc.allow_low_precision(reason="fc2 spk/W2 in fp16, fp32 accum"):
                for b in range(NB):
                    sl = slice(b * (J // NB), (b + 1) * (J // NB))
                    nc.tensor.matmul(
                        yp[:, sl], s2t[:], spkb[:, sl], start=True, stop=True
                    )
                # ACT: + b2 and evict PSUM -> SBUF
                nc.scalar.activation(
                    yt[:], yp[:], AF.Identity, bias=b2t[:], scale=1.0
                )

            nc.sync.dma_start(y_d[:], yt[:])
    return nc


_CACHE = {}


def _get_program():
    if "nc" not in _CACHE:
        nc = bacc.Bacc("TRN2", target_bir_lowering=False, debug=False,
                       num_devices=N_CORES)
        build(nc)
        nc.compile()
        _CACHE["nc"] = nc
    return _CACHE["nc"]


def _make_consts(W1, b1, W2, b2):
    s1 = np.zeros((KM, P), dtype=np.float32)
    for i in range(N_IN):
        for r4 in range(NG):
            s1[i * NG + r4, r4 * N_HID : (r4 + 1) * N_HID] = -W1[:, i]
    for r4 in range(NG):
        s1[KM - 1, r4 * N_HID : (r4 + 1) * N_HID] = -b1
    s2 = np.zeros((P, QO), dtype=np.float16)
    for o in range(N_OUT):
        for r4 in range(NG):
            s2[r4 * N_HID : (r4 + 1) * N_HID, o * NG + r4] = W2[o].astype(
                np.float16
            )
    b2c = np.zeros((QO, 1), dtype=np.float32)
    for o in range(N_OUT):
        for r4 in range(NG):
            b2c[o * NG + r4, 0] = b2[o]
    return s1, s2, b2c


def kernel(x, W1, b1, W2, b2):
    x = np.asarray(x, dtype=np.float32)
    W1, b1, W2, b2 = (np.asarray(a, dtype=np.float32) for a in (W1, b1, W2, b2))
    s1, s2, b2c = _make_consts(W1, b1, W2, b2)
    nc = _get_program()
    in_maps = []
    for i in range(N_CORES):
        xs = x[i * BC : (i + 1) * BC].reshape(NG, J, N_IN)
        xj = np.empty((KM, J), dtype=np.float32)
        xj[: N_IN * NG] = xs.transpose(2, 0, 1).reshape(N_IN * NG, J)
        xj[KM - 1] = 1.0
        in_maps.append({"xj": xj, "s1": s1, "s2": s2, "b2c": b2c})
    kwargs = dict(_CACHE.get("run_kwargs") or {})
    res = run_bass_kernel_spmd(nc, in_maps, core_ids=list(range(N_CORES)), **kwargs)
    _CACHE["last_results"] = res
    # y[(o*NG+r4), j] <-> out[r4*J + j, o]
    out = np.empty((B, N_OUT), dtype=np.float32)
    for i in range(N_CORES):
        yc = res.results[i]["y"].reshape(N_OUT, NG, J)
        out[i * BC : (i + 1) * BC] = yc.transpose(1, 2, 0).reshape(BC, N_OUT)
    return out
